# revision 4
# baseline (speedup 1.0000x reference)
"""Distributed GCN encoder (2x spmm+linear) on 8 Trainium2 NeuronCores — v2.

Strategy (v2): partition destination nodes contiguously across the 8 cores;
each core owns the edges whose destination is local. spmm and the dense
Linear commute, so each layer is: dense projection -> gather projected rows
per edge -> one-hot-matmul segment reduce in PSUM.

v2 changes vs v1:
- bf16 gather tables, messages, and one-hot S matrices; matmuls run at
  1 cyc/row instead of fp32's 4.
- Layer-1 projection (x@W1) is computed replicated on every core (cheap at
  bf16 rates), removing the first AllGather and its barrier; phase A is
  ordered by gather-chunk so layer-1 gathers start while later chunks are
  still being projected.
- Layer-2 table (h1@W2) is bf16 padded to 128 cols so the 256B-min gather
  constraint holds; matmuls slice the real 64 cols.
- dma_gather spread round-robin over SWDGE queues (K_NSWQ, default 4).
"""
import os
import sys

sys.path.insert(0, "/opt/trn_rl_repo")

import numpy as np

NCORES = 8
CHUNK_MAX = 25000  # dma_gather idx is int16; chunk the gather table
BLK = 128          # dest nodes per PSUM block (= matmul N)
SBB = 4            # blocks per superblock (PSUM bank = 512 f32)

LAST_RESULT = None  # BassKernelResults of the most recent run (for test.py)


def kernel(x, adj_rows, adj_cols, adj_vals, W1, b1, W2, b2):
    return _run(
        np.asarray(x, np.float32),
        np.asarray(adj_rows, np.int32),
        np.asarray(adj_cols, np.int32),
        np.asarray(adj_vals, np.float32),
        np.asarray(W1, np.float32),
        np.asarray(b1, np.float32),
        np.asarray(W2, np.float32),
        np.asarray(b2, np.float32),
    )


def _pack_idx16(idx):
    # dma_gather idxs layout: linear k -> [16*g + k%16, k//16], replicated
    # across the 8 groups of 16 partitions so any SWDGE queue's Q7 pair
    # reads its copy.
    n = idx.shape[0]
    a = idx.astype(np.int16).reshape(n // 16, 16).T
    return np.tile(a, (8, 1))


def _preprocess(rows, cols, vals, N, NLOC, NBLK, NCHUNK, CHUNK):
    """Sort/pad edges into the shared (superblock, chunk, block) tile grid."""
    NSB = (NBLK + SBB - 1) // SBB
    core = rows // NLOC
    rloc = rows - core * NLOC
    blk = rloc // BLK
    ch = cols // CHUNK
    gid = blk * NCHUNK + ch  # group id

    # global group order: sb-major, then chunk, then block
    order_pos = np.empty(NBLK * NCHUNK, np.int64)
    seq = []
    for s in range(NSB):
        bs = range(s * SBB, min((s + 1) * SBB, NBLK))
        for c in range(NCHUNK):
            for b in bs:
                seq.append(b * NCHUNK + c)
    seq = np.array(seq, np.int64)
    order_pos[seq] = np.arange(len(seq))

    # per-core group counts -> shared tile counts
    counts = np.zeros((NCORES, NBLK * NCHUNK), np.int64)
    for c in range(NCORES):
        m = core == c
        counts[c] = np.bincount(gid[m], minlength=NBLK * NCHUNK)
    T = (counts.max(0) + BLK - 1) // BLK  # tiles per group (shared)

    T_seq = T[seq]
    tile_base_seq = np.concatenate([[0], np.cumsum(T_seq)])
    NT = int(tile_base_seq[-1])
    tile_base = np.empty(NBLK * NCHUNK, np.int64)
    tile_base[seq] = tile_base_seq[:-1]

    per_core = []
    for c in range(NCORES):
        m = core == c
        ec, er, ev, eg = cols[m], rloc[m], vals[m], gid[m]
        o = np.lexsort((ec, order_pos[eg]))
        ec, er, ev, eg = ec[o], er[o], ev[o], eg[o]
        cnt = counts[c]
        starts_per_group = np.concatenate([[0], np.cumsum(cnt[seq])])[:-1]
        g_start = np.empty(NBLK * NCHUNK, np.int64)
        g_start[seq] = starts_per_group
        rank = np.arange(len(ec)) - g_start[eg]
        slot = tile_base[eg] * BLK + rank

        idx_arr = np.zeros(NT * BLK, np.int16)
        rows_arr = np.full(NT * BLK, -1.0, np.float32)
        vals_arr = np.zeros(NT * BLK, np.float32)
        idx_arr[slot] = (ec - (ec // CHUNK) * CHUNK).astype(np.int16)
        if os.environ.get("K_SEQ_IDX"):  # perf ablation: coalesced gather reads
            idx_arr = (np.arange(NT * BLK) % 12500).astype(np.int16)
        rows_arr[slot] = (er - (er // BLK) * BLK).astype(np.float32)
        vals_arr[slot] = ev
        per_core.append(
            (
                _pack_idx16(idx_arr),
                np.ascontiguousarray(rows_arr.reshape(NT, BLK).T).astype(np.float16),
                np.ascontiguousarray(vals_arr.reshape(NT, BLK).T).astype(np.float16),
            )
        )
    return T, tile_base, NT, per_core


def _bf16(a):
    import jax.numpy as jnp

    return np.asarray(jnp.asarray(a, dtype=jnp.bfloat16))


def _run(x, adj_rows, adj_cols, adj_vals, W1, b1, W2, b2, trace=None):
    global LAST_RESULT
    import concourse.bacc as bacc
    import concourse.mybir as mybir
    import concourse.tile as tile
    from concourse import bass_utils
    from concourse.bass import broadcast_tensor_aps

    N, F0 = x.shape
    F1 = W1.shape[1]
    F2 = W2.shape[1]
    E = adj_rows.shape[0]
    assert N % NCORES == 0
    NLOC = N // NCORES
    NBLK = (NLOC + BLK - 1) // BLK
    NSB = (NBLK + SBB - 1) // SBB
    NCHUNK = (N + CHUNK_MAX - 1) // CHUNK_MAX
    CHUNK = (N + NCHUNK - 1) // NCHUNK
    assert CHUNK <= 32768

    T, tile_base, NT, per_core = _preprocess(
        adj_rows, adj_cols, adj_vals, N, NLOC, NBLK, NCHUNK, CHUNK
    )

    iota_np = np.tile(np.arange(BLK, dtype=np.float32)[None, :], (BLK, 1)).astype(
        np.float16
    )
    b2bc_np = np.tile(b2[None, :], (BLK, SBB))

    f32 = mybir.dt.float32
    bf16 = mybir.dt.bfloat16
    nc = bacc.Bacc(
        "TRN2",
        target_bir_lowering=False,
        debug=False,
        num_devices=NCORES,
        num_swdge_queues=int(os.environ.get("K_NSWQ", "4")),
    )
    NSWQ = nc.num_swdge_queues

    # xT2[p, j, i] = x[i, p + 128*j] as bf16 (full table, replicated)
    xT2_t = nc.dram_tensor("xT2", [128, F0 // 128, N], bf16, kind="ExternalInput")
    W1_t = nc.dram_tensor("W1", [F0, F1], bf16, kind="ExternalInput")
    b1_t = nc.dram_tensor("b1", [F1, 1], f32, kind="ExternalInput")
    W2_t = nc.dram_tensor("W2", [F1, F2], bf16, kind="ExternalInput")
    b2bc_t = nc.dram_tensor("b2bc", [BLK, SBB * F2], f32, kind="ExternalInput")
    iota_t = nc.dram_tensor("iota", [BLK, BLK], bf16, kind="ExternalInput")
    idx_t = nc.dram_tensor("idx16", [128, NT * 8], mybir.dt.int16, kind="ExternalInput")
    rows_t = nc.dram_tensor("rowsT", [128, NT], bf16, kind="ExternalInput")
    vals_t = nc.dram_tensor("valsT", [128, NT], bf16, kind="ExternalInput")
    out_t = nc.dram_tensor("out", [NLOC, F2], f32, kind="ExternalOutput")

    xw1_full = nc.dram_tensor("xw1_full", [N, F1], bf16, kind="Internal")
    F2P = 128  # layer-2 table padded to 128 cols for the 256B gather minimum
    h1w2_bounce = nc.dram_tensor("h1w2_bounce", [NLOC, F2P], bf16, kind="Internal")
    h1w2_full = nc.dram_tensor(
        "h1w2_full", [N, F2P], bf16, kind="Internal", addr_space="Shared"
    )

    # per-superblock static structure
    sb_blocks = [list(range(s * SBB, min((s + 1) * SBB, NBLK))) for s in range(NSB)]
    sb_base = []  # first global tile of each sb
    sb_nt = []
    for s in range(NSB):
        gids = [b * NCHUNK + c for c in range(NCHUNK) for b in sb_blocks[s]]
        bases = [tile_base[g] for g in gids if T[g] > 0]
        nt = int(sum(T[g] for g in gids))
        sb_base.append(int(min(bases)) if bases else 0)
        sb_nt.append(nt)
    MAXNT = max(sb_nt) if sb_nt else 1

    is_eq = mybir.AluOpType.is_equal
    mult = mybir.AluOpType.mult
    add = mybir.AluOpType.add
    Relu = mybir.ActivationFunctionType.Relu
    REPEAT = int(os.environ.get("K_REPEAT", "1"))

    with tile.TileContext(nc) as tc:
        with (
            tc.tile_pool(name="consts", bufs=1) as cp,
            tc.tile_pool(name="xt", bufs=3) as xtp,
            tc.tile_pool(name="xw1sb", bufs=3) as xw1p,
            tc.tile_pool(name="edata", bufs=2) as ep,
            tc.tile_pool(name="msgs", bufs=4) as mp,
            tc.tile_pool(name="smat", bufs=2) as sp,
            tc.tile_pool(name="epi", bufs=2) as epi,
            tc.tile_pool(name="ps_a", bufs=2, space="PSUM") as ppa,
            tc.tile_pool(name="ps_agg", bufs=2, space="PSUM") as ppagg,
            tc.tile_pool(name="ps_w2", bufs=2, space="PSUM") as ppw2,
            tc.tile_pool(name="ps_l2", bufs=2, space="PSUM") as ppl2,
        ):
            iota_sb = cp.tile([BLK, BLK], bf16, tag="iota")
            nc.sync.dma_start(iota_sb[:], iota_t.ap())
            w1_sb = cp.tile([128, (F0 // 128) * F1], bf16, tag="w1")
            for k in range(F0 // 128):
                nc.sync.dma_start(
                    w1_sb[:, k * F1 : (k + 1) * F1], W1_t[k * 128 : (k + 1) * 128, :]
                )
            w2_sb = cp.tile([F1, F2], bf16, tag="w2")
            nc.sync.dma_start(w2_sb[:], W2_t.ap())
            b1_sb = cp.tile([F1, 1], f32, tag="b1")
            nc.sync.dma_start(b1_sb[:], b1_t.ap())
            b2_sb = cp.tile([BLK, SBB * F2], f32, tag="b2bc")
            nc.sync.dma_start(b2_sb[:], b2bc_t.ap())

            for _rep in range(REPEAT):
                # ---- phase A: xw1_full = x @ W1, computed replicated on every
                # core, ordered by gather chunk so L1 gathers pipeline behind it.
                # 256-row loads keep per-partition DMA runs at 512B.
                AROWS = 2 * BLK
                NTA = (N + AROWS - 1) // AROWS
                for i in range(NTA):
                    nr = min(AROWS, N - i * AROWS)
                    xt = xtp.tile([128, F0 // 128, AROWS], bf16, tag="xt")
                    nc.sync.dma_start(
                        xt[:, :, :nr], xT2_t[:, :, i * AROWS : i * AROWS + nr]
                    )
                    for h in range(0, nr, BLK):
                        nb_r = min(BLK, nr - h)
                        ps = ppa.tile([128, F1], f32, tag="psa")
                        for k in range(F0 // 128):
                            nc.tensor.matmul(
                                ps[:nb_r, :],
                                xt[:, k, h : h + nb_r],
                                w1_sb[:, k * F1 : (k + 1) * F1],
                                start=(k == 0),
                                stop=(k == F0 // 128 - 1),
                            )
                        xo = xw1p.tile([128, F1], bf16, tag="xw1")
                        nc.scalar.copy(xo[:nb_r, :], ps[:nb_r, :])
                        nc.sync.dma_start(
                            xw1_full[i * AROWS + h : i * AROWS + h + nb_r, :],
                            xo[:nb_r, :],
                        )

                # ---- phase B: layer-1 gather + segment-reduce + relu + @W2
                def edge_tiles(s):
                    idx_sb = ep.tile([128, MAXNT * 8], mybir.dt.int16, tag="idx")
                    rows_sb = ep.tile([128, MAXNT], bf16, tag="rows")
                    vals_sb = ep.tile([128, MAXNT], bf16, tag="vals")
                    base, nt = sb_base[s], sb_nt[s]
                    nc.sync.dma_start(idx_sb[:, : nt * 8], idx_t[:, base * 8 : (base + nt) * 8])
                    nc.sync.dma_start(rows_sb[:, :nt], rows_t[:, base : base + nt])
                    nc.sync.dma_start(vals_sb[:, :nt], vals_t[:, base : base + nt])
                    return idx_sb, rows_sb, vals_sb

                TSC_MAX = max(
                    max(
                        int(sum(T[b * NCHUNK + c] for b in sb_blocks[s]))
                        for c in range(NCHUNK)
                    )
                    for s in range(NSB)
                )

                def gather_chunk(s, c, off, tsc, msgs_c, idx_sb, table, F):
                    n = tsc * BLK
                    lo = c * CHUNK
                    hi = min((c + 1) * CHUNK, N)
                    if os.environ.get("K_STUB_GATHER"):
                        nc.sync.dma_start(
                            msgs_c[:, :tsc, :],
                            table[lo : lo + n, :].rearrange("(t p) f -> p t f", p=128),
                        )
                        return
                    nc.gpsimd.dma_gather(
                        msgs_c[:, :tsc, :],
                        table[lo:hi, :],
                        idx_sb[:, off * 8 : (off + tsc) * 8],
                        n,
                        n,
                        F,
                        single_packet=bool(int(os.environ.get("K_SP", "0"))),
                        queue_num=(s * NCHUNK + c) % NSWQ,
                    )

                def build_S(off, tsc, rows_sb, vals_sb):
                    S = sp.tile([128, TSC_MAX, BLK], bf16, tag="S")
                    i_bc, r_bc = broadcast_tensor_aps(
                        iota_sb[:][:, None, :], rows_sb[:, off : off + tsc][:, :, None]
                    )
                    nc.vector.tensor_tensor(S[:, :tsc, :], i_bc, r_bc, is_eq)
                    s_ap, v_bc = broadcast_tensor_aps(
                        S[:, :tsc, :], vals_sb[:, off : off + tsc][:, :, None]
                    )
                    nc.vector.tensor_tensor(S[:, :tsc, :], s_ap, v_bc, mult)
                    return S

                def aggregate(s, idx_sb, rows_sb, vals_sb, table, F, Fm, psum, pw, mtag):
                    """Per-chunk gather + S build + one-hot matmuls into psum.

                    F: gathered row width; Fm: matmul width (<= F);
                    pw: psum free-dim width per block; lhsT/rhs roles flip
                    between layers (transposed agg for L1, plain for L2)."""
                    tot = {
                        b: int(sum(T[b * NCHUNK + c] for c in range(NCHUNK)))
                        for b in sb_blocks[s]
                    }
                    done = {b: 0 for b in sb_blocks[s]}
                    # Interleaved per-slice start groups would lazily re-zero the
                    # whole 2KB bank and clobber sibling slices; instead memset
                    # once and accumulate with start=False throughout.
                    nc.vector.memset(psum[:], 0.0)
                    off = 0
                    for c in range(NCHUNK):
                        tsc = int(sum(T[b * NCHUNK + c] for b in sb_blocks[s]))
                        if tsc == 0:
                            continue
                        msgs_c = mp.tile([128, TSC_MAX, F], bf16, tag=mtag)
                        gather_chunk(s, c, off, tsc, msgs_c, idx_sb, table, F)
                        S = build_S(off, tsc, rows_sb, vals_sb)
                        run0 = tile_base[sb_blocks[s][0] * NCHUNK + c]
                        for bi, b in enumerate(sb_blocks[s]):
                            g = b * NCHUNK + c
                            t0 = int(tile_base[g] - run0)
                            dst = psum[:, bi * pw : (bi + 1) * pw]
                            for j in range(int(T[g])):
                                lt = t0 + j
                                if pw == BLK:  # L1: psum[f1, d] = msgs.T @ S
                                    lhsT, rhs = msgs_c[:, lt, :], S[:, lt, :]
                                else:  # L2: psum[d, f2] = S.T @ msgs
                                    lhsT, rhs = S[:, lt, :], msgs_c[:, lt, :Fm]
                                nc.tensor.matmul(
                                    dst,
                                    lhsT,
                                    rhs,
                                    start=False,
                                    stop=(done[b] == tot[b] - 1),
                                    skip_group_check=True,
                                )
                                done[b] += 1
                        off += tsc

                for s in range(NSB):
                    nb = len(sb_blocks[s])
                    idx_sb, rows_sb, vals_sb = edge_tiles(s)
                    psum1 = ppagg.tile([128, SBB * BLK], f32, tag="agg")
                    aggregate(
                        s, idx_sb, rows_sb, vals_sb, xw1_full, F1, F1, psum1, BLK, "msgs"
                    )

                    h1T = epi.tile([128, SBB * BLK], bf16, tag="h1T")
                    nc.scalar.activation(
                        h1T[:, : nb * BLK], psum1[:, : nb * BLK], Relu, bias=b1_sb[:, 0:1]
                    )
                    psum2 = ppw2.tile([128, SBB * F2], f32, tag="w2out")
                    for bi in range(nb):
                        nc.tensor.matmul(
                            psum2[:, bi * F2 : (bi + 1) * F2],
                            h1T[:, bi * BLK : (bi + 1) * BLK],
                            w2_sb[:],
                            start=True,
                            stop=True,
                        )
                    hw = epi.tile([128, SBB * F2P], bf16, tag="hw2")
                    nc.vector.memset(hw[:, : nb * F2P], 0.0)
                    for bi in range(nb):
                        nc.scalar.copy(
                            hw[:, bi * F2P : bi * F2P + F2],
                            psum2[:, bi * F2 : (bi + 1) * F2],
                        )
                    for bi, b in enumerate(sb_blocks[s]):
                        nr = min(BLK, NLOC - b * BLK)
                        nc.sync.dma_start(
                            h1w2_bounce[b * BLK : b * BLK + nr, :],
                            hw[:nr, bi * F2P : (bi + 1) * F2P],
                        )

                if "ag" not in os.environ.get("K_SKIP", ""):
                    nc.gpsimd.collective_compute(
                        "AllGather",
                        mybir.AluOpType.bypass,
                        replica_groups=[list(range(NCORES))],
                        ins=[h1w2_bounce.ap()],
                        outs=[h1w2_full.ap()],
                    )

                # ---- phase C: layer-2 gather + segment-reduce + bias
                for s in ([] if "l2" in os.environ.get("K_SKIP", "") else range(NSB)):
                    nb = len(sb_blocks[s])
                    idx_sb, rows_sb, vals_sb = edge_tiles(s)
                    psum3 = ppl2.tile([128, SBB * F2], f32, tag="l2agg")
                    aggregate(
                        s, idx_sb, rows_sb, vals_sb, h1w2_full, F2P, F2, psum3, F2, "msgs2"
                    )

                    osb = epi.tile([128, SBB * F2], f32, tag="osb")
                    nc.vector.tensor_tensor(
                        osb[:, : nb * F2], psum3[:, : nb * F2], b2_sb[:, : nb * F2], add
                    )
                    for bi, b in enumerate(sb_blocks[s]):
                        nr = min(BLK, NLOC - b * BLK)
                        nc.sync.dma_start(
                            out_t[b * BLK : b * BLK + nr, :],
                            osb[:nr, bi * F2 : (bi + 1) * F2],
                        )

    nc.compile()

    import jax.numpy as jnp

    xT2 = np.ascontiguousarray(
        np.transpose(x.reshape(N, F0 // 128, 128), (2, 1, 0))
    )
    xT2 = np.asarray(jnp.asarray(xT2, dtype=jnp.bfloat16))
    W1b = np.asarray(jnp.asarray(W1, dtype=jnp.bfloat16))
    W2b = np.asarray(jnp.asarray(W2, dtype=jnp.bfloat16))
    iota_b = np.asarray(jnp.asarray(iota_np, dtype=jnp.bfloat16))

    in_maps = []
    for c in range(NCORES):
        idx16, rowsT, valsT = per_core[c]
        in_maps.append(
            {
                "xT2": xT2,
                "W1": W1b,
                "b1": np.ascontiguousarray(b1[:, None]),
                "W2": W2b,
                "b2bc": b2bc_np,
                "iota": iota_b,
                "idx16": idx16,
                "rowsT": np.asarray(jnp.asarray(rowsT, dtype=jnp.bfloat16)),
                "valsT": np.asarray(jnp.asarray(valsT, dtype=jnp.bfloat16)),
            }
        )

    bench = int(os.environ.get("K_BENCH", "0"))
    if bench:
        results = _pjrt_bench(nc, in_maps, bench)
    else:
        kwargs = {}
        if trace is not None:
            kwargs["trace"] = trace
        res = bass_utils.run_bass_kernel_spmd(
            nc, in_maps, core_ids=list(range(NCORES)), **kwargs
        )
        LAST_RESULT = res
        results = res.results
    return np.concatenate([results[c]["out"] for c in range(NCORES)], axis=0)


LAST_TIMES = None


def _pjrt_bench(nc, in_maps, iters):
    """Replicates bass2jax.run_bass_via_pjrt's multi-core path, with the
    executable built once and timed warm iterations (inputs pre-staged on
    device, zero output-donation buffers made on device)."""
    global LAST_TIMES
    import time

    import jax
    import jax.numpy as jnp
    from jax.sharding import Mesh, NamedSharding, PartitionSpec
    from jax.experimental.shard_map import shard_map

    import concourse.mybir as mybir
    from concourse.bass2jax import (
        _bass_exec_p,
        install_neuronx_cc_hook,
        partition_id_tensor,
    )

    install_neuronx_cc_hook()

    in_names, out_names, out_avals, zero_outs = [], [], [], []
    partition_name = nc.partition_id_tensor.name if nc.partition_id_tensor else None
    for alloc in nc.m.functions[0].allocations:
        if not isinstance(alloc, mybir.MemoryLocationSet):
            continue
        name = alloc.memorylocations[0].name
        if alloc.kind == "ExternalInput":
            if name != partition_name:
                in_names.append(name)
        elif alloc.kind == "ExternalOutput":
            out_names.append(name)
            shape = tuple(alloc.tensor_shape)
            dtype = mybir.dt.np(alloc.dtype)
            out_avals.append(jax.core.ShapedArray(shape, dtype))
            zero_outs.append(np.zeros(shape, dtype))
    n_params = len(in_names)
    n_outs = len(out_avals)
    in_names.extend(out_names)
    if partition_name is not None:
        in_names.append(partition_name)

    def _make_body(chain):
        def _exec(zs, ins):
            operands = ins + list(zs)
            if partition_name is not None:
                operands.append(partition_id_tensor())
            return tuple(
                _bass_exec_p.bind(
                    *operands,
                    out_avals=tuple(out_avals),
                    in_names=tuple(in_names),
                    out_names=tuple(out_names),
                    lowering_input_output_aliases=(),
                    sim_require_finite=True,
                    sim_require_nnan=True,
                    nc=nc,
                )
            )

        def _body(*args):
            ins = list(args[:n_params])
            zs = tuple(args[n_params:])
            if chain == 1:
                return _exec(zs, ins)
            return jax.lax.fori_loop(0, chain, lambda i, z: _exec(z, ins), zs)

        return _body

    ncores = len(in_maps)
    devices = jax.devices()[:ncores]
    mesh = Mesh(np.asarray(devices), ("core",))
    donate = tuple(range(n_params, n_params + n_outs))

    def _make_sharded(chain):
        return jax.jit(
            shard_map(
                _make_body(chain),
                mesh=mesh,
                in_specs=(PartitionSpec("core"),) * (n_params + n_outs),
                out_specs=(PartitionSpec("core"),) * n_outs,
                check_rep=False,
            ),
            donate_argnums=donate,
            keep_unused=True,
        )

    sharded = _make_sharded(1)
    sh = NamedSharding(mesh, PartitionSpec("core"))
    concat_in = [
        np.concatenate([np.asarray(m[in_names[i]]) for m in in_maps], axis=0)
        for i in range(n_params)
    ]
    dev_in = [jax.device_put(a, sh) for a in concat_in]
    zshapes = [(ncores * z.shape[0], *z.shape[1:]) for z in zero_outs]
    zdtypes = [z.dtype for z in zero_outs]
    zeros_maker = jax.jit(
        lambda: tuple(jnp.zeros(s, d) for s, d in zip(zshapes, zdtypes)),
        out_shardings=(sh,) * n_outs,
    )

    def _time_fn(fn, n):
        out, ts = None, []
        for i in range(n + 1):
            zs = jax.block_until_ready(zeros_maker())
            t0 = time.perf_counter()
            cur = jax.block_until_ready(fn(*dev_in, *zs))
            dt = time.perf_counter() - t0
            if i > 0:
                ts.append(dt)
            else:
                out = cur
        return out, ts

    out_arrs, t1 = _time_fn(sharded, iters)
    chain = int(os.environ.get("K_CHAIN", "1"))
    tc = []
    if chain > 1:
        _, tc = _time_fn(_make_sharded(chain), iters)
        per_exec = (min(tc) - min(t1)) / (chain - 1)
        print(
            f"bench: chain1 {min(t1)*1e3:.2f} ms, chain{chain} {min(tc)*1e3:.2f} ms"
            f" -> per-exec {per_exec*1e3:.3f} ms"
        )
    LAST_TIMES = {"t1": t1, "tc": tc, "chain": chain}
    return [
        {
            name: np.asarray(out_arrs[i]).reshape(ncores, *out_avals[i].shape)[c]
            for i, name in enumerate(out_names)
        }
        for c in range(ncores)
    ]
